# revision 1
# baseline (speedup 1.0000x reference)
"""AdaLN Trainium2 kernel v5 — raw Bass, replicated weights, tuned engines.

Measured-cost-driven design (from kernel3/kernel4 HW traces):
  - collectives have ~100us fixed latency here -> replicate the 8MB w read
  - scalar_tensor_tensor is ~3x slower/elem than tensor_scalar ->
    modulation is ts (x*inv, AP scalar) + tt (*a) instead of one stt
  - explicit DVE drains cost the preceding op's pipe flush -> one per iter
  - GPSIMD tensor_tensor ~2.3us -> it owns the '+shift' pass
  - conditioning quant chunks split DVE/GPSIMD, pipelined with per-chunk
    weight DMAs and matmuls

x path per [128,1024] tile i:
  SP   : DMA in
  ACT  : square+accum -> ss ; drain ; sqrt(ss/D+eps) -> std  [inc std]
  DVE  : t = x*inv_prev (ts) ; recip(next std) ; drain ; y = t*a (tt) [inc mod]
  GPS  : y += b (tt)  [inc add]
  SP   : DMA out
"""

import sys
from contextlib import ExitStack

import numpy as np

sys.path.insert(0, "/opt/trn_rl_repo")
sys.path.insert(0, "/opt/pypackages")

import concourse.bass as bass
from concourse import mybir
from concourse.bass_utils import run_bass_kernel_spmd

F32 = mybir.dt.float32
BF16 = mybir.dt.bfloat16
ALU = mybir.AluOpType
ACTF = mybir.ActivationFunctionType

P = 128
D = 1024
CD = 1024
DD = 2 * D
B = 8
S_FULL = 4096

EPS_RMS = 1e-6
EPS_Q = 1e-5
MAGIC = 1.5 * 2.0**23
WCLIP = 1.25


def build(S=S_FULL, NX=12, NY=5, NS=10, NWQ=4):
    nc = bass.Bass()

    x_d = nc.declare_dram_parameter("x", [S, D], F32, isOutput=False)
    c_d = nc.declare_dram_parameter("c", [CD], F32, isOutput=False)
    wt_d = nc.declare_dram_parameter("wt", [CD, DD], F32, isOutput=False)
    b_d = nc.declare_dram_parameter("b", [DD], F32, isOutput=False)
    g_d = nc.declare_dram_parameter("g", [D], F32, isOutput=False)
    out_d = nc.declare_dram_parameter("out", [S, D], F32, isOutput=True)

    KC = CD // P
    NT = S // P
    NX = min(NX, NT)
    NY = min(NY, NT)
    NS = min(NS, NT)

    ctx = ExitStack()
    with ctx:
        # ---------------- SBUF ----------------
        ones = ctx.enter_context(nc.sbuf_tensor("ones", [P, P], F32))
        eps_t = ctx.enter_context(nc.sbuf_tensor("eps", [P, 1], F32))
        wt_sb = ctx.enter_context(nc.sbuf_tensor("wt_sb", [P, KC, DD], F32))
        red = ctx.enter_context(nc.sbuf_tensor("red", [P, KC], F32))
        sw = ctx.enter_context(nc.sbuf_tensor("sw", [P, 1], F32))
        swa = ctx.enter_context(nc.sbuf_tensor("swa", [P, 1], F32))
        mp = ctx.enter_context(nc.sbuf_tensor("mp", [P, 1], F32))
        swinv = ctx.enter_context(nc.sbuf_tensor("swinv", [P, 1], F32))
        tqD = [
            ctx.enter_context(nc.sbuf_tensor(f"tqD{j}", [P, DD], F32))
            for j in range(2)
        ]
        wq = [
            ctx.enter_context(nc.sbuf_tensor(f"wq{j}", [P, DD], BF16))
            for j in range(NWQ)
        ]
        c_row = ctx.enter_context(nc.sbuf_tensor("c_row", [1, CD], F32))
        ct = ctx.enter_context(nc.sbuf_tensor("ct", [P, KC], F32))
        am = ctx.enter_context(nc.sbuf_tensor("am", [1, 1], F32))
        amc = ctx.enter_context(nc.sbuf_tensor("amc", [1, 1], F32))
        rc = ctx.enter_context(nc.sbuf_tensor("rc", [1, 1], F32))
        r127 = ctx.enter_context(nc.sbuf_tensor("r127", [1, 1], F32))
        r127_b = ctx.enter_context(nc.sbuf_tensor("r127_b", [P, 1], F32))
        cqt = ctx.enter_context(nc.sbuf_tensor("cqt", [P, KC], F32))
        cqi = ctx.enter_context(nc.sbuf_tensor("cqi", [P, KC], BF16))
        osx = ctx.enter_context(nc.sbuf_tensor("osx", [1, 1], F32))
        os_t = ctx.enter_context(nc.sbuf_tensor("os_t", [1, 1], F32))
        b_row = ctx.enter_context(nc.sbuf_tensor("b_row", [1, DD], F32))
        g_row = ctx.enter_context(nc.sbuf_tensor("g_row", [1, D], F32))
        emb = ctx.enter_context(nc.sbuf_tensor("emb", [1, DD], F32))
        a_row = ctx.enter_context(nc.sbuf_tensor("a_row", [1, D], F32))
        a_bc = ctx.enter_context(nc.sbuf_tensor("a_bc", [P, D], F32))
        b_bc = ctx.enter_context(nc.sbuf_tensor("b_bc", [P, D], F32))

        xt = [
            ctx.enter_context(nc.sbuf_tensor(f"xt{j}", [P, D], F32))
            for j in range(NX)
        ]
        yt = [
            ctx.enter_context(nc.sbuf_tensor(f"yt{j}", [P, D], F32))
            for j in range(NY)
        ]
        sq = ctx.enter_context(nc.sbuf_tensor("sqs", [P, D], F32))
        ss = [
            ctx.enter_context(nc.sbuf_tensor(f"ss{j}", [P, 1], F32))
            for j in range(NS)
        ]
        std = [
            ctx.enter_context(nc.sbuf_tensor(f"std{j}", [P, 1], F32))
            for j in range(NS)
        ]
        inv = [
            ctx.enter_context(nc.sbuf_tensor(f"inv{j}", [P, 1], F32))
            for j in range(NS)
        ]

        par_ps = ctx.enter_context(nc.psum_tensor("par_ps", [P, 2], F32))
        emb_ps = ctx.enter_context(nc.psum_tensor("emb_ps", [1, 4, 512], F32))
        bc_ps = [
            ctx.enter_context(nc.psum_tensor(f"bc_ps{j}", [P, 512], F32))
            for j in range(2)
        ]

        # ---------------- semaphores ----------------
        sem_pre = ctx.enter_context(nc.semaphore("pre"))
        sem_wk = [ctx.enter_context(nc.semaphore(f"wk{k}")) for k in range(KC)]
        sem_vec = ctx.enter_context(nc.semaphore("vec"))
        sem_red = ctx.enter_context(nc.semaphore("reds"))
        sem_sw = ctx.enter_context(nc.semaphore("sws"))
        sem_swcp = ctx.enter_context(nc.semaphore("swcp"))
        sem_pe1 = ctx.enter_context(nc.semaphore("pe1"))
        sem_qrdy = ctx.enter_context(nc.semaphore("qrdy"))   # DVE: swinv ready
        sem_wqD = ctx.enter_context(nc.semaphore("wqD"))
        sem_wqG = ctx.enter_context(nc.semaphore("wqG"))
        sem_r127 = ctx.enter_context(nc.semaphore("r127s"))
        sem_r127cp = ctx.enter_context(nc.semaphore("r127cp"))
        sem_cq = ctx.enter_context(nc.semaphore("cqs"))
        sem_mmk = ctx.enter_context(nc.semaphore("mmk"))     # PE per-chunk mm
        sem_emb = ctx.enter_context(nc.semaphore("embs"))
        sem_bcmm = ctx.enter_context(nc.semaphore("bcmm"))
        sem_bccp = ctx.enter_context(nc.semaphore("bccp"))
        sem_std = ctx.enter_context(nc.semaphore("stds"))
        sem_mod = ctx.enter_context(nc.semaphore("mods"))
        sem_add = ctx.enter_context(nc.semaphore("adds"))
        sem_xt = [ctx.enter_context(nc.semaphore(f"xs{j}")) for j in range(NX)]
        sem_ot = [ctx.enter_context(nc.semaphore(f"ot{j}")) for j in range(NY)]

        wt_r = wt_d[:].rearrange("(k p) n -> k p n", p=P)
        # chunk -> quantizing engine: even chunks on DVE, odd on GPSIMD
        qeng = ["D" for k in range(KC)]
        nqD = [0] * KC
        nqG = [0] * KC
        cD = cG = 0
        for k in range(KC):
            if qeng[k] == "D":
                cD += 1
            else:
                cG += 1
            nqD[k], nqG[k] = cD, cG

        with nc.Block() as block:

            # ================= SP =================
            @block.sync
            def _(sync):
                sync.dma_start(out=c_row[:], in_=c_d[None, :]).then_inc(sem_vec, 16)
                with nc.allow_non_contiguous_dma(reason="tiny 4KB c chunk load"):
                    sync.dma_start(
                        out=ct[:], in_=c_d[:].rearrange("(k p) -> p k", p=P)
                    ).then_inc(sem_vec, 16)
                sync.dma_start(out=b_row[:], in_=b_d[None, :]).then_inc(sem_vec, 16)
                sync.dma_start(out=g_row[:], in_=g_d[None, :]).then_inc(sem_vec, 16)
                for k in range(KC):
                    sync.dma_start(out=wt_sb[:, k, :], in_=wt_r[k, :, :]).then_inc(
                        sem_wk[k], 16
                    )
                for j in range(NX):
                    sync.dma_start(
                        out=xt[j][:], in_=x_d[j * P : (j + 1) * P, :]
                    ).then_inc(sem_xt[j], 16)
                for i in range(NT):
                    sync.wait_ge(sem_add, i + 1)
                    sync.dma_start(
                        out=out_d[i * P : (i + 1) * P, :], in_=yt[i % NY][:]
                    ).then_inc(sem_ot[i % NY], 16)
                    if i + NX < NT:
                        j = i + NX
                        sync.dma_start(
                            out=xt[j % NX][:], in_=x_d[j * P : (j + 1) * P, :]
                        ).then_inc(sem_xt[j % NX], 16)
                for j in range(NY):
                    cnt = (NT - j + NY - 1) // NY
                    sync.wait_ge(sem_ot[j], 16 * cnt)

            # ================= DVE =================
            @block.vector
            def _(vector):
                vector.memset(ones[:], 1.0).then_inc(sem_pre, 1)
                vector.memset(eps_t[:], EPS_RMS).then_inc(sem_pre, 1)

                # --- c quant (early: only needs the tiny vec DMAs + PE bcast) ---
                vector.wait_ge(sem_vec, 64)
                vector.tensor_reduce(
                    out=am[:], in_=c_row[:], axis=mybir.AxisListType.X,
                    op=ALU.max, apply_absolute_value=True,
                )
                vector.drain()
                vector.tensor_scalar(
                    out=amc[:], in0=am[:], scalar1=EPS_Q, scalar2=None, op0=ALU.max
                )
                vector.drain()
                vector.reciprocal(rc[:], amc[:])
                vector.drain()
                vector.tensor_scalar(
                    out=r127[:], in0=rc[:], scalar1=127.0, scalar2=None,
                    op0=ALU.mult,
                ).then_inc(sem_r127, 1)
                vector.wait_ge(sem_pe1, 1)
                vector.tensor_copy(r127_b[:], par_ps[:, 1:2]).then_inc(sem_r127cp, 1)
                vector.drain()
                vector.tensor_scalar(
                    out=cqt[:], in0=ct[:], scalar1=r127_b[:], scalar2=MAGIC,
                    op0=ALU.mult, op1=ALU.add,
                )
                vector.drain()
                vector.tensor_scalar(
                    out=cqi[:], in0=cqt[:], scalar1=MAGIC, scalar2=None,
                    op0=ALU.subtract,
                ).then_inc(sem_cq, 1)

                # --- weight stats: per-chunk reduces as DMAs land ---
                for k in range(KC):
                    vector.wait_ge(sem_wk[k], 16)
                    vector.tensor_reduce(
                        out=red[:, k : k + 1],
                        in_=wt_sb[:, k, :],
                        axis=mybir.AxisListType.X,
                        op=ALU.add,
                        apply_absolute_value=True,
                    )
                vector.drain()
                vector.tensor_reduce(
                    out=sw[:], in_=red[:], axis=mybir.AxisListType.X, op=ALU.add
                ).then_inc(sem_sw, 1)
                vector.wait_ge(sem_pe1, 2)
                vector.tensor_copy(swa[:], par_ps[:, 0:1]).then_inc(sem_swcp, 1)
                vector.drain()
                vector.tensor_scalar(
                    out=mp[:], in0=swa[:], scalar1=1.0 / (CD * DD), scalar2=EPS_Q,
                    op0=ALU.mult, op1=ALU.max,
                )
                vector.drain()
                vector.reciprocal(swinv[:], mp[:]).then_inc(sem_qrdy, 1)
                vector.drain()

                # --- DVE's half of the w quant (even chunks) ---
                for k in range(KC):
                    if qeng[k] != "D":
                        continue
                    if k >= NWQ:
                        vector.wait_ge(sem_mmk, k - NWQ + 1)
                    vector.tensor_scalar(
                        out=tqD[k % 2][:], in0=wt_sb[:, k, :], scalar1=swinv[:],
                        scalar2=WCLIP, op0=ALU.mult, op1=ALU.min,
                    )
                    vector.drain()
                    vector.tensor_scalar(
                        out=tqD[k % 2][:], in0=tqD[k % 2][:], scalar1=-WCLIP,
                        scalar2=MAGIC, op0=ALU.max, op1=ALU.add,
                    )
                    vector.drain()
                    vector.tensor_scalar(
                        out=wq[k % NWQ][:], in0=tqD[k % 2][:], scalar1=MAGIC,
                        scalar2=None, op0=ALU.subtract,
                    ).then_inc(sem_wqD, 1)

                # --- output scale ---
                vector.tensor_tensor(osx[:], amc[:], mp[0:1, :], op=ALU.mult)
                vector.drain()
                vector.tensor_scalar(
                    out=os_t[:], in0=osx[:], scalar1=1.0 / 127.0, scalar2=None,
                    op0=ALU.mult,
                )

                # --- emb epilogue ---
                vector.wait_ge(sem_mmk, KC)
                vector.drain()
                for n in range(4):
                    sl = slice(n * 512, (n + 1) * 512)
                    vector.scalar_tensor_tensor(
                        out=emb[:, sl], in0=emb_ps[:, n, :], scalar=os_t[:],
                        in1=b_row[:, sl], op0=ALU.mult, op1=ALU.add,
                    )
                vector.drain()
                vector.scalar_tensor_tensor(
                    out=a_row[:], in0=emb[:, 0:D], scalar=1.0, in1=g_row[:],
                    op0=ALU.add, op1=ALU.mult,
                ).then_inc(sem_emb, 1)

                # --- x pipeline: pure stt stream (inv comes from ACT ln/exp) ---
                vector.wait_ge(sem_bccp, 4)
                vector.drain()
                for j in range(NT):
                    vector.wait_ge(sem_std, j + 1)
                    if j >= NY:
                        vector.wait_ge(sem_ot[j % NY], 16 * (j // NY))
                    vector.scalar_tensor_tensor(
                        out=yt[j % NY][:], in0=xt[j % NX][:],
                        scalar=inv[j % NS][:], in1=a_bc[:],
                        op0=ALU.mult, op1=ALU.mult,
                    ).then_inc(sem_mod, 1)

            # ================= ACT =================
            @block.scalar
            def _(scalar):
                scalar.wait_ge(sem_pre, 2)
                copy_at = min(8, NT - 1)
                done_copies = False

                def bcast_copies():
                    for j in range(4):
                        scalar.wait_ge(sem_bcmm, j + 1)
                        half = j % 2
                        sl = slice(half * 512, (half + 1) * 512)
                        dstt = a_bc if j < 2 else b_bc
                        scalar.copy(dstt[:, sl], bc_ps[half][:, :]).then_inc(
                            sem_bccp, 1
                        )

                for i in range(NT):
                    if i == copy_at:
                        bcast_copies()
                        done_copies = True
                    scalar.wait_ge(sem_xt[i % NX], 16 * (i // NX + 1))
                    if i >= NS:
                        # ss/lg/inv slots must be consumed first
                        scalar.wait_ge(sem_mod, i - NS + 1)
                    scalar.drain()
                    scalar.activation(
                        sq[:], xt[i % NX][:], ACTF.Square,
                        accum_out=ss[i % NS][:],
                    )
                    if i >= 1:
                        scalar.activation(
                            std[(i - 1) % NS][:], ss[(i - 1) % NS][:], ACTF.Ln,
                            bias=eps_t[:], scale=1.0 / D,
                        )
                    if i >= 2:
                        scalar.activation(
                            inv[(i - 2) % NS][:], std[(i - 2) % NS][:], ACTF.Exp,
                            scale=-0.5,
                        ).then_inc(sem_std, 1)
                for i in (NT, NT + 1):
                    scalar.drain()
                    if i - 1 < NT:
                        scalar.activation(
                            std[(i - 1) % NS][:], ss[(i - 1) % NS][:], ACTF.Ln,
                            bias=eps_t[:], scale=1.0 / D,
                        )
                    scalar.activation(
                        inv[(i - 2) % NS][:], std[(i - 2) % NS][:], ACTF.Exp,
                        scale=-0.5,
                    ).then_inc(sem_std, 1)
                if not done_copies:
                    bcast_copies()

            # ================= GPSIMD =================
            @block.gpsimd
            def _(gpsimd):
                gpsimd.wait_ge(sem_bccp, 4)
                for i in range(NT):
                    gpsimd.wait_ge(sem_mod, i + 1)
                    gpsimd.tensor_tensor(
                        out=yt[i % NY][:], in0=yt[i % NY][:], in1=b_bc[:],
                        op=ALU.add,
                    ).then_inc(sem_add, 1)

            # ================= PE =================
            @block.tensor
            def _(tensor):
                tensor.wait_ge(sem_pre, 1)
                tensor.wait_ge(sem_r127, 1)
                tensor.matmul(
                    par_ps[:, 1:2], lhsT=ones[0:1, :], rhs=r127[:],
                    start=True, stop=True,
                ).then_inc(sem_pe1, 1)
                tensor.wait_ge(sem_r127cp, 1)
                tensor.wait_ge(sem_sw, 1)
                tensor.matmul(
                    par_ps[:, 0:1], lhsT=ones[:], rhs=sw[:], start=True, stop=True
                ).then_inc(sem_pe1, 1)
                tensor.wait_ge(sem_cq, 1)
                for k in range(KC):
                    if qeng[k] == "D":
                        tensor.wait_ge(sem_wqD, nqD[k])
                    else:
                        tensor.wait_ge(sem_wqG, nqG[k])
                    for n in range(4):
                        mmi = tensor.matmul(
                            emb_ps[:, n, :],
                            lhsT=cqi[:, k : k + 1],
                            rhs=wq[k % NWQ][:, n * 512 : (n + 1) * 512],
                            start=(k == 0),
                            stop=(k == KC - 1),
                        )
                        if n == 3:
                            mmi.then_inc(sem_mmk, 1)
                tensor.wait_ge(sem_emb, 1)
                tensor.matmul(
                    bc_ps[0][:], lhsT=ones[0:1, :], rhs=a_row[:, 0:512],
                    start=True, stop=True,
                ).then_inc(sem_bcmm, 1)
                tensor.matmul(
                    bc_ps[1][:], lhsT=ones[0:1, :], rhs=a_row[:, 512:1024],
                    start=True, stop=True,
                ).then_inc(sem_bcmm, 1)
                tensor.wait_ge(sem_bccp, 2)
                tensor.matmul(
                    bc_ps[0][:], lhsT=ones[0:1, :], rhs=emb[:, D : D + 512],
                    start=True, stop=True,
                ).then_inc(sem_bcmm, 1)
                tensor.matmul(
                    bc_ps[1][:], lhsT=ones[0:1, :], rhs=emb[:, D + 512 : DD],
                    start=True, stop=True,
                ).then_inc(sem_bcmm, 1)

    return nc


_CACHE = {}


def _built(S=S_FULL):
    key = ("nc", S)
    if key not in _CACHE:
        _CACHE[key] = build(S)
    return _CACHE[key]


def kernel(x, c, w_proj, b_proj, rms_weight, _trace=False):
    x = np.ascontiguousarray(np.asarray(x, dtype=np.float32))
    c = np.ascontiguousarray(np.asarray(c, dtype=np.float32))
    w_proj = np.asarray(w_proj, dtype=np.float32)
    b_proj = np.ascontiguousarray(np.asarray(b_proj, dtype=np.float32))
    rms_weight = np.ascontiguousarray(np.asarray(rms_weight, dtype=np.float32))

    nc = _built(x.shape[1])
    wt = np.ascontiguousarray(w_proj.T)

    in_maps = [
        {"x": x[i], "c": c[i], "wt": wt, "b": b_proj, "g": rms_weight}
        for i in range(B)
    ]
    res = run_bass_kernel_spmd(nc, in_maps, list(range(B)), trace=_trace)
    kernel.last_results = res
    kernel.last_exec_time_ns = res.exec_time_ns
    return np.stack([res.results[i]["out"] for i in range(B)], axis=0)



# revision 4
# speedup vs baseline: 1.1423x; 1.1423x over previous
"""AdaLN Trainium2 kernel v6 — raw Bass, replicated weights.

Redesign from the v5 trace (199.4us): the conditioning path was ~88us of
serial work (w DMA behind x prefetch, 3-pass f32 w-quant on DVE) and the
x-pipeline used a 3.16us/tile stt.  v6:

  - DMA priority: w chunks stream FIRST at full BW (done ~24us), x tiles
    behind them; out DMAs interleave as tiles complete.
  - w-quant via bf16-write rounding: ACT computes Copy(w*swinv + 192) ->
    bf16 (ULP=1 in [128,256) => RNE integer snap), DVE clamps [191,193]
    and subtracts 192 in two 4x-mode ts passes (0.53us each).
  - weight |w| chunk reduces on ACT (Abs + accum_out), off DVE.
  - modulation per tile: z = x*inv (DVE ts f32->bf16), y = z*A (DVE tt
    bf16 2x), out = y+B (f32 out; even tiles on GPSIMD, odd on DVE).
  - in-place x buffers (load -> square -> z -> y -> +B -> store).

x path per [128,1024] tile i:
  SP  : DMA in (ring NX)
  ACT : square+accum -> ss ; ln ; exp -> inv
  DVE : z = x*inv (bf16) ; y = z*a_bc (bf16)
  DVE/GPS (alternating): out = y + b_bc (f32, in-place into x buffer)
  SP  : DMA out
"""

import sys
from contextlib import ExitStack

import numpy as np

sys.path.insert(0, "/opt/trn_rl_repo")
sys.path.insert(0, "/opt/pypackages")

import concourse.bass as bass
from concourse import mybir
from concourse.bass_utils import run_bass_kernel_spmd

F32 = mybir.dt.float32
BF16 = mybir.dt.bfloat16
ALU = mybir.AluOpType
ACTF = mybir.ActivationFunctionType

P = 128
D = 1024
CD = 1024
DD = 2 * D
B = 8
S_FULL = 4096

EPS_RMS = 1e-6
EPS_Q = 1e-5
MAGIC = 1.5 * 2.0**23  # f32 round-to-int trick (c quant)
MBF = 192.0            # bf16 round-to-int magic (w quant): ULP=1 in [128,256)
CHI = 193.0
CLO = 191.0


def build(S=S_FULL, NX=18, NZ=6, NU=4):
    nc = bass.Bass()

    x_d = nc.declare_dram_parameter("x", [S, D], F32, isOutput=False)
    c_d = nc.declare_dram_parameter("c", [CD], F32, isOutput=False)
    wt_d = nc.declare_dram_parameter("wt", [CD, DD], F32, isOutput=False)
    b_d = nc.declare_dram_parameter("b", [DD], F32, isOutput=False)
    g_d = nc.declare_dram_parameter("g", [D], F32, isOutput=False)
    out_d = nc.declare_dram_parameter("out", [S, D], F32, isOutput=True)

    KC = CD // P
    NT = S // P
    NX = min(NX, NT)
    NS = min(24, NT)

    # tile ownership of the final "+shift" pass: even -> GPSIMD, odd -> DVE
    def owner_is_gps(i):
        return i % 2 == 0

    def n_addG(j):  # count of GPS-owned tiles <= j
        return j // 2 + 1

    def n_addD(j):  # count of DVE-owned tiles <= j
        return (j + 1) // 2

    ctx = ExitStack()
    with ctx:
        # ---------------- SBUF ----------------
        ones = ctx.enter_context(nc.sbuf_tensor("ones", [P, P], F32))
        eps_t = ctx.enter_context(nc.sbuf_tensor("eps", [P, 1], F32))
        wt_sb = ctx.enter_context(nc.sbuf_tensor("wt_sb", [P, KC, DD], F32))
        red = ctx.enter_context(nc.sbuf_tensor("red", [P, KC], F32))
        sw = ctx.enter_context(nc.sbuf_tensor("sw", [P, 1], F32))
        swa = ctx.enter_context(nc.sbuf_tensor("swa", [P, 1], F32))
        mp = ctx.enter_context(nc.sbuf_tensor("mp", [P, 1], F32))
        swinv = ctx.enter_context(nc.sbuf_tensor("swinv", [P, 1], F32))
        u = [
            ctx.enter_context(nc.sbuf_tensor(f"u{j}", [P, DD], BF16))
            for j in range(NU)
        ]
        c_row = ctx.enter_context(nc.sbuf_tensor("c_row", [1, CD], F32))
        ct = ctx.enter_context(nc.sbuf_tensor("ct", [P, KC], F32))
        am = ctx.enter_context(nc.sbuf_tensor("am", [1, 1], F32))
        amc = ctx.enter_context(nc.sbuf_tensor("amc", [1, 1], F32))
        rc = ctx.enter_context(nc.sbuf_tensor("rc", [1, 1], F32))
        r127 = ctx.enter_context(nc.sbuf_tensor("r127", [1, 1], F32))
        r127_b = ctx.enter_context(nc.sbuf_tensor("r127_b", [P, 1], F32))
        cqt = ctx.enter_context(nc.sbuf_tensor("cqt", [P, KC], F32))
        cqi = ctx.enter_context(nc.sbuf_tensor("cqi", [P, KC], BF16))
        osx = ctx.enter_context(nc.sbuf_tensor("osx", [1, 1], F32))
        os_t = ctx.enter_context(nc.sbuf_tensor("os_t", [1, 1], F32))
        b_row = ctx.enter_context(nc.sbuf_tensor("b_row", [1, DD], F32))
        g_row = ctx.enter_context(nc.sbuf_tensor("g_row", [1, D], F32))
        emb = ctx.enter_context(nc.sbuf_tensor("emb", [1, DD], F32))
        a_row = ctx.enter_context(nc.sbuf_tensor("a_row", [1, D], F32))
        a_bc = ctx.enter_context(nc.sbuf_tensor("a_bc", [P, D], BF16))
        b_bc = ctx.enter_context(nc.sbuf_tensor("b_bc", [P, D], BF16))

        xt = [
            ctx.enter_context(nc.sbuf_tensor(f"xt{j}", [P, D], F32))
            for j in range(NX)
        ]
        zt = [
            ctx.enter_context(nc.sbuf_tensor(f"zt{j}", [P, D], BF16))
            for j in range(NZ)
        ]
        sq = ctx.enter_context(nc.sbuf_tensor("sqs", [P, D], F32))
        ss = [
            ctx.enter_context(nc.sbuf_tensor(f"ss{j}", [P, 1], F32))
            for j in range(NS)
        ]
        std = [
            ctx.enter_context(nc.sbuf_tensor(f"std{j}", [P, 1], F32))
            for j in range(NS)
        ]
        inv = [
            ctx.enter_context(nc.sbuf_tensor(f"inv{j}", [P, 1], F32))
            for j in range(NS)
        ]

        par_ps = ctx.enter_context(nc.psum_tensor("par_ps", [P, 2], F32))
        emb_ps = ctx.enter_context(nc.psum_tensor("emb_ps", [1, 4, 512], F32))
        bc_ps = [
            ctx.enter_context(nc.psum_tensor(f"bc_ps{j}", [P, 512], F32))
            for j in range(2)
        ]

        # ---------------- semaphores ----------------
        sem_pre = ctx.enter_context(nc.semaphore("pre"))
        sem_vec = ctx.enter_context(nc.semaphore("vec"))
        sem_wk = ctx.enter_context(nc.semaphore("wk"))
        sem_xt = ctx.enter_context(nc.semaphore("xts"))
        sem_ot = ctx.enter_context(nc.semaphore("ots"))
        sem_r127 = ctx.enter_context(nc.semaphore("r127s"))
        sem_r127cp = ctx.enter_context(nc.semaphore("r127cp"))
        sem_pe1 = ctx.enter_context(nc.semaphore("pe1"))
        sem_cq = ctx.enter_context(nc.semaphore("cqs"))
        sem_redA = ctx.enter_context(nc.semaphore("redA"))
        sem_sw = ctx.enter_context(nc.semaphore("sws"))
        sem_qrdy = ctx.enter_context(nc.semaphore("qrdy"))
        sem_mg = ctx.enter_context(nc.semaphore("mg"))
        sem_wq = ctx.enter_context(nc.semaphore("wq"))
        sem_mmk = ctx.enter_context(nc.semaphore("mmk"))
        sem_emb = ctx.enter_context(nc.semaphore("embs"))
        sem_embB = ctx.enter_context(nc.semaphore("embB"))
        sem_bcmm = ctx.enter_context(nc.semaphore("bcmm"))
        sem_bccp = ctx.enter_context(nc.semaphore("bccp"))
        sem_std = ctx.enter_context(nc.semaphore("stds"))
        sem_mod = ctx.enter_context(nc.semaphore("mods"))
        sem_addD = ctx.enter_context(nc.semaphore("addD"))
        sem_addG = ctx.enter_context(nc.semaphore("addG"))

        wt_r = wt_d[:].rearrange("(k p) n -> k p n", p=P)

        def add_wait(eng, j):
            # wait until tile j's "+shift" has completed (owner-class count)
            if owner_is_gps(j):
                eng.wait_ge(sem_addG, n_addG(j))
            else:
                eng.wait_ge(sem_addD, n_addD(j))

        with nc.Block() as block:

            # ================= SP =================
            @block.sync
            def _(sync):
                sync.dma_start(out=c_row[:], in_=c_d[None, :]).then_inc(sem_vec, 16)
                with nc.allow_non_contiguous_dma(reason="tiny 4KB c chunk load"):
                    sync.dma_start(
                        out=ct[:], in_=c_d[:].rearrange("(k p) -> p k", p=P)
                    ).then_inc(sem_vec, 16)
                sync.dma_start(out=b_row[:], in_=b_d[None, :]).then_inc(sem_vec, 16)
                sync.dma_start(out=g_row[:], in_=g_d[None, :]).then_inc(sem_vec, 16)
                # weight chunks at full priority, then the x ring
                for k in range(KC):
                    sync.dma_start(out=wt_sb[:, k, :], in_=wt_r[k, :, :]).then_inc(
                        sem_wk, 16
                    )
                for j in range(NX):
                    sync.dma_start(
                        out=xt[j][:], in_=x_d[j * P : (j + 1) * P, :]
                    ).then_inc(sem_xt, 16)
                for i in range(NT):
                    add_wait(sync, i)
                    sync.dma_start(
                        out=out_d[i * P : (i + 1) * P, :], in_=xt[i % NX][:]
                    ).then_inc(sem_ot, 16)
                    if i + NX < NT:
                        j = i + NX
                        sync.dma_start(
                            out=xt[j % NX][:], in_=x_d[j * P : (j + 1) * P, :]
                        ).then_inc(sem_xt, 16)
                sync.wait_ge(sem_ot, 16 * NT)

            # ================= ACT =================
            @block.scalar
            def _(scalar):
                scalar.wait_ge(sem_pre, 2)
                # per-chunk |w| reduces as the weight DMAs land
                for k in range(KC):
                    scalar.wait_ge(sem_wk, 16 * (k + 1))
                    scalar.drain()
                    scalar.activation(
                        u[k % NU][:], wt_sb[:, k, :], ACTF.Abs,
                        accum_out=red[:, k : k + 1],
                    ).then_inc(sem_redA, 1)
                # magic-round pass: u = bf16(w*swinv + 192)
                scalar.wait_ge(sem_qrdy, 1)
                for k in range(KC):
                    if k >= NU:
                        scalar.wait_ge(sem_mmk, k - NU + 1)
                    scalar.drain()
                    scalar.activation(
                        u[k % NU][:], wt_sb[:, k, :], ACTF.Copy,
                        bias=MBF, scale=swinv[:],
                    ).then_inc(sem_mg, 1)
                # x statistics stream
                for i in range(NT):
                    scalar.wait_ge(sem_xt, 16 * (i + 1))
                    if i >= NS:
                        scalar.wait_ge(sem_mod, i - NS + 1)
                    scalar.drain()
                    scalar.activation(
                        sq[:], xt[i % NX][:], ACTF.Square,
                        accum_out=ss[i % NS][:],
                    )
                    if i >= 1:
                        scalar.activation(
                            std[(i - 1) % NS][:], ss[(i - 1) % NS][:], ACTF.Ln,
                            bias=eps_t[:], scale=1.0 / D,
                        )
                    if i >= 2:
                        scalar.activation(
                            inv[(i - 2) % NS][:], std[(i - 2) % NS][:], ACTF.Exp,
                            scale=-0.5,
                        ).then_inc(sem_std, 1)
                for i in (NT, NT + 1):
                    scalar.drain()
                    if i - 1 < NT:
                        scalar.activation(
                            std[(i - 1) % NS][:], ss[(i - 1) % NS][:], ACTF.Ln,
                            bias=eps_t[:], scale=1.0 / D,
                        )
                    scalar.activation(
                        inv[(i - 2) % NS][:], std[(i - 2) % NS][:], ACTF.Exp,
                        scale=-0.5,
                    ).then_inc(sem_std, 1)

            # ================= DVE =================
            @block.vector
            def _(vector):
                vector.memset(ones[:], 1.0).then_inc(sem_pre, 1)
                vector.memset(eps_t[:], EPS_RMS).then_inc(sem_pre, 1)

                # --- c quant ---
                vector.wait_ge(sem_vec, 64)
                vector.tensor_reduce(
                    out=am[:], in_=c_row[:], axis=mybir.AxisListType.X,
                    op=ALU.max, apply_absolute_value=True,
                )
                vector.drain()
                vector.tensor_scalar(
                    out=amc[:], in0=am[:], scalar1=EPS_Q, scalar2=None, op0=ALU.max
                )
                vector.drain()
                vector.reciprocal(rc[:], amc[:])
                vector.drain()
                vector.tensor_scalar(
                    out=r127[:], in0=rc[:], scalar1=127.0, scalar2=None,
                    op0=ALU.mult,
                ).then_inc(sem_r127, 1)
                vector.wait_ge(sem_pe1, 1)
                vector.tensor_copy(r127_b[:], par_ps[:, 1:2]).then_inc(sem_r127cp, 1)
                vector.drain()
                vector.tensor_scalar(
                    out=cqt[:], in0=ct[:], scalar1=r127_b[:], scalar2=MAGIC,
                    op0=ALU.mult, op1=ALU.add,
                )
                vector.drain()
                vector.tensor_scalar(
                    out=cqi[:], in0=cqt[:], scalar1=MAGIC, scalar2=None,
                    op0=ALU.subtract,
                ).then_inc(sem_cq, 1)

                # --- weight stats tail ---
                vector.wait_ge(sem_redA, KC)
                vector.tensor_reduce(
                    out=sw[:], in_=red[:], axis=mybir.AxisListType.X, op=ALU.add
                ).then_inc(sem_sw, 1)
                vector.wait_ge(sem_pe1, 2)
                vector.tensor_copy(swa[:], par_ps[:, 0:1])
                vector.drain()
                vector.tensor_scalar(
                    out=mp[:], in0=swa[:], scalar1=1.0 / (CD * DD), scalar2=EPS_Q,
                    op0=ALU.mult, op1=ALU.max,
                )
                vector.drain()
                vector.reciprocal(swinv[:], mp[:]).then_inc(sem_qrdy, 1)
                vector.drain()

                # --- output scale ---
                vector.tensor_tensor(osx[:], amc[:], mp[0:1, :], op=ALU.mult)
                vector.drain()
                vector.tensor_scalar(
                    out=os_t[:], in0=osx[:], scalar1=1.0 / 127.0, scalar2=None,
                    op0=ALU.mult,
                )

                # --- w quant: clamp to [191,193], subtract 192 -> {-1,0,1} ---
                for k in range(KC):
                    vector.wait_ge(sem_mg, k + 1)
                    vector.tensor_scalar(
                        out=u[k % NU][:], in0=u[k % NU][:], scalar1=CHI,
                        scalar2=CLO, op0=ALU.min, op1=ALU.max,
                    )
                    vector.drain()
                    vector.tensor_scalar(
                        out=u[k % NU][:], in0=u[k % NU][:], scalar1=MBF,
                        scalar2=None, op0=ALU.subtract,
                    ).then_inc(sem_wq, 1)

                # --- emb epilogue (scale half first, then shift half) ---
                vector.wait_ge(sem_mmk, KC)
                vector.drain()
                vector.scalar_tensor_tensor(
                    out=emb[:, 0:512], in0=emb_ps[:, 0, :], scalar=os_t[:],
                    in1=b_row[:, 0:512], op0=ALU.mult, op1=ALU.add,
                )
                vector.scalar_tensor_tensor(
                    out=emb[:, 512:1024], in0=emb_ps[:, 1, :], scalar=os_t[:],
                    in1=b_row[:, 512:1024], op0=ALU.mult, op1=ALU.add,
                )
                vector.drain()
                vector.scalar_tensor_tensor(
                    out=a_row[:, 0:512], in0=emb[:, 0:512], scalar=1.0,
                    in1=g_row[:, 0:512], op0=ALU.add, op1=ALU.mult,
                ).then_inc(sem_emb, 1)
                vector.scalar_tensor_tensor(
                    out=a_row[:, 512:1024], in0=emb[:, 512:1024], scalar=1.0,
                    in1=g_row[:, 512:1024], op0=ALU.add, op1=ALU.mult,
                ).then_inc(sem_emb, 1)
                vector.scalar_tensor_tensor(
                    out=emb[:, 1024:1536], in0=emb_ps[:, 2, :], scalar=os_t[:],
                    in1=b_row[:, 1024:1536], op0=ALU.mult, op1=ALU.add,
                ).then_inc(sem_embB, 1)
                vector.scalar_tensor_tensor(
                    out=emb[:, 1536:2048], in0=emb_ps[:, 3, :], scalar=os_t[:],
                    in1=b_row[:, 1536:2048], op0=ALU.mult, op1=ALU.add,
                ).then_inc(sem_embB, 1)

                # --- broadcast copies PSUM -> SBUF (bf16) ---
                vector.wait_ge(sem_bcmm, 1)
                vector.tensor_copy(a_bc[:, 0:512], bc_ps[0][:]).then_inc(sem_bccp, 1)
                vector.wait_ge(sem_bcmm, 2)
                vector.tensor_copy(a_bc[:, 512:1024], bc_ps[1][:]).then_inc(
                    sem_bccp, 1
                )
                vector.wait_ge(sem_bcmm, 3)
                vector.tensor_copy(b_bc[:, 0:512], bc_ps[0][:]).then_inc(sem_bccp, 1)
                vector.wait_ge(sem_bcmm, 4)
                vector.tensor_copy(b_bc[:, 512:1024], bc_ps[1][:]).then_inc(
                    sem_bccp, 1
                )
                vector.drain()

                # --- x modulation stream ---
                for i in range(NT):
                    vector.wait_ge(sem_std, i + 1)
                    if i >= NZ:
                        add_wait(vector, i - NZ)
                    vector.tensor_scalar(
                        out=zt[i % NZ][:], in0=xt[i % NX][:],
                        scalar1=inv[i % NS][:], scalar2=None, op0=ALU.mult,
                    )
                    vector.drain()
                    vector.tensor_tensor(
                        out=zt[i % NZ][:], in0=zt[i % NZ][:], in1=a_bc[:],
                        op=ALU.mult,
                    ).then_inc(sem_mod, 1)
                    if not owner_is_gps(i):
                        vector.drain()
                        vector.tensor_tensor(
                            out=xt[i % NX][:], in0=zt[i % NZ][:], in1=b_bc[:],
                            op=ALU.add,
                        ).then_inc(sem_addD, 1)

            # ================= GPSIMD =================
            @block.gpsimd
            def _(gpsimd):
                gpsimd.wait_ge(sem_bccp, 4)  # b_bc ready
                for i in range(NT):
                    if not owner_is_gps(i):
                        continue
                    gpsimd.wait_ge(sem_mod, i + 1)
                    gpsimd.tensor_tensor(
                        out=xt[i % NX][:], in0=zt[i % NZ][:], in1=b_bc[:],
                        op=ALU.add,
                    ).then_inc(sem_addG, 1)

            # ================= PE =================
            @block.tensor
            def _(tensor):
                tensor.wait_ge(sem_pre, 1)
                tensor.wait_ge(sem_r127, 1)
                tensor.matmul(
                    par_ps[:, 1:2], lhsT=ones[0:1, :], rhs=r127[:],
                    start=True, stop=True,
                ).then_inc(sem_pe1, 1)
                tensor.wait_ge(sem_r127cp, 1)
                tensor.wait_ge(sem_sw, 1)
                tensor.matmul(
                    par_ps[:, 0:1], lhsT=ones[:], rhs=sw[:], start=True, stop=True
                ).then_inc(sem_pe1, 1)
                tensor.wait_ge(sem_cq, 1)
                for k in range(KC):
                    tensor.wait_ge(sem_wq, k + 1)
                    for n in range(4):
                        mmi = tensor.matmul(
                            emb_ps[:, n, :],
                            lhsT=cqi[:, k : k + 1],
                            rhs=u[k % NU][:, n * 512 : (n + 1) * 512],
                            start=(k == 0),
                            stop=(k == KC - 1),
                        )
                        if n == 3:
                            mmi.then_inc(sem_mmk, 1)
                tensor.wait_ge(sem_emb, 1)
                tensor.matmul(
                    bc_ps[0][:], lhsT=ones[0:1, :], rhs=a_row[:, 0:512],
                    start=True, stop=True,
                ).then_inc(sem_bcmm, 1)
                tensor.wait_ge(sem_emb, 2)
                tensor.matmul(
                    bc_ps[1][:], lhsT=ones[0:1, :], rhs=a_row[:, 512:1024],
                    start=True, stop=True,
                ).then_inc(sem_bcmm, 1)
                tensor.wait_ge(sem_bccp, 2)
                tensor.wait_ge(sem_embB, 1)
                tensor.matmul(
                    bc_ps[0][:], lhsT=ones[0:1, :], rhs=emb[:, D : D + 512],
                    start=True, stop=True,
                ).then_inc(sem_bcmm, 1)
                tensor.wait_ge(sem_embB, 2)
                tensor.matmul(
                    bc_ps[1][:], lhsT=ones[0:1, :], rhs=emb[:, D + 512 : DD],
                    start=True, stop=True,
                ).then_inc(sem_bcmm, 1)

    return nc


_CACHE = {}


def _built(S=S_FULL):
    key = ("nc", S)
    if key not in _CACHE:
        _CACHE[key] = build(S)
    return _CACHE[key]


def kernel(x, c, w_proj, b_proj, rms_weight, _trace=False):
    x = np.ascontiguousarray(np.asarray(x, dtype=np.float32))
    c = np.ascontiguousarray(np.asarray(c, dtype=np.float32))
    w_proj = np.asarray(w_proj, dtype=np.float32)
    b_proj = np.ascontiguousarray(np.asarray(b_proj, dtype=np.float32))
    rms_weight = np.ascontiguousarray(np.asarray(rms_weight, dtype=np.float32))

    nc = _built(x.shape[1])
    wt = np.ascontiguousarray(w_proj.T)

    in_maps = [
        {"x": x[i], "c": c[i], "wt": wt, "b": b_proj, "g": rms_weight}
        for i in range(B)
    ]
    res = run_bass_kernel_spmd(nc, in_maps, list(range(B)), trace=_trace)
    kernel.last_results = res
    kernel.last_exec_time_ns = res.exec_time_ns
    return np.stack([res.results[i]["out"] for i in range(B)], axis=0)


# revision 5
# speedup vs baseline: 1.3576x; 1.1885x over previous
"""AdaLN Trainium2 kernel v7 — raw Bass, replicated weights.

From the v6 trace (174.6us): GPSIMD shares SBUF ports with DVE, so
concurrent GPS adds slowed DVE ~3.5x; the SP engine spends ~1us of
descriptor-generation per dma_start, so small/gather DMAs issued before
the weight chunks delayed them ~10us; ACT's packed magic passes gated
both the quant cadence and the square stream.  v7:

  - GPSIMD idle (its port contention costs more than its work).
  - SP issues the 8 weight-chunk DMAs FIRST, then small rows, then x.
  - w-quant magic (bf16-write RNE round) split: ACT even chunks, DVE
    odd chunks; DVE clamps+subtracts all chunks (4x bf16 ts passes).
  - modulation all on DVE (z=x*inv ts bf16, y=z*A tt bf16, out=y+B tt
    f32 in-place) with ACT computing z for tiles i%3!=0 to offload.

x path per [128,1024] tile i:
  SP  : DMA in (ring NX)
  ACT : square+accum -> ss ; ln ; exp -> inv ; (z for i%3!=0)
  DVE : (z for i%3==0) ; y = z*a_bc (bf16) ; out = y + b_bc (f32)
  SP  : DMA out
"""

import sys
from contextlib import ExitStack

import numpy as np

sys.path.insert(0, "/opt/trn_rl_repo")
sys.path.insert(0, "/opt/pypackages")

import concourse.bass as bass
from concourse import mybir
from concourse.bass_utils import run_bass_kernel_spmd

F32 = mybir.dt.float32
BF16 = mybir.dt.bfloat16
ALU = mybir.AluOpType
ACTF = mybir.ActivationFunctionType

P = 128
D = 1024
CD = 1024
DD = 2 * D
B = 8
S_FULL = 4096

EPS_RMS = 1e-6
EPS_Q = 1e-5
MAGIC = 1.5 * 2.0**23  # f32 round-to-int trick (c quant)
MBF = 192.0            # bf16 round-to-int magic (w quant): ULP=1 in [128,256)
CHI = 193.0
CLO = 191.0


def build(S=S_FULL, NX=18, NZ=6, NU=4):
    nc = bass.Bass()

    x_d = nc.declare_dram_parameter("x", [S, D], F32, isOutput=False)
    c_d = nc.declare_dram_parameter("c", [CD], F32, isOutput=False)
    wt_d = nc.declare_dram_parameter("wt", [CD, DD], F32, isOutput=False)
    b_d = nc.declare_dram_parameter("b", [DD], F32, isOutput=False)
    g_d = nc.declare_dram_parameter("g", [D], F32, isOutput=False)
    out_d = nc.declare_dram_parameter("out", [S, D], F32, isOutput=True)

    KC = CD // P
    NT = S // P
    NX = min(NX, NT)
    NS = min(24, NT)

    def act_owns_z(i):
        return i % 3 != 0

    def n_zA(j):  # count of ACT-z tiles <= j
        return sum(1 for t in range(j + 1) if act_owns_z(t))

    def act_magic(k):
        return k % 2 == 0

    def n_mg(k):  # count of ACT-magic chunks <= k
        return sum(1 for t in range(k + 1) if act_magic(t))

    ctx = ExitStack()
    with ctx:
        # ---------------- SBUF ----------------
        ones = ctx.enter_context(nc.sbuf_tensor("ones", [P, P], F32))
        eps_t = ctx.enter_context(nc.sbuf_tensor("eps", [P, 1], F32))
        wt_sb = ctx.enter_context(nc.sbuf_tensor("wt_sb", [P, KC, DD], F32))
        red = ctx.enter_context(nc.sbuf_tensor("red", [P, KC], F32))
        sw = ctx.enter_context(nc.sbuf_tensor("sw", [P, 1], F32))
        swa = ctx.enter_context(nc.sbuf_tensor("swa", [P, 1], F32))
        mp = ctx.enter_context(nc.sbuf_tensor("mp", [P, 1], F32))
        swinv = ctx.enter_context(nc.sbuf_tensor("swinv", [P, 1], F32))
        u = [
            ctx.enter_context(nc.sbuf_tensor(f"u{j}", [P, DD], BF16))
            for j in range(NU)
        ]
        c_row = ctx.enter_context(nc.sbuf_tensor("c_row", [1, CD], F32))
        ct = ctx.enter_context(nc.sbuf_tensor("ct", [P, KC], F32))
        am = ctx.enter_context(nc.sbuf_tensor("am", [1, 1], F32))
        amc = ctx.enter_context(nc.sbuf_tensor("amc", [1, 1], F32))
        rc = ctx.enter_context(nc.sbuf_tensor("rc", [1, 1], F32))
        r127 = ctx.enter_context(nc.sbuf_tensor("r127", [1, 1], F32))
        r127_b = ctx.enter_context(nc.sbuf_tensor("r127_b", [P, 1], F32))
        cqt = ctx.enter_context(nc.sbuf_tensor("cqt", [P, KC], F32))
        cqi = ctx.enter_context(nc.sbuf_tensor("cqi", [P, KC], BF16))
        osx = ctx.enter_context(nc.sbuf_tensor("osx", [1, 1], F32))
        os_t = ctx.enter_context(nc.sbuf_tensor("os_t", [1, 1], F32))
        b_row = ctx.enter_context(nc.sbuf_tensor("b_row", [1, DD], F32))
        g_row = ctx.enter_context(nc.sbuf_tensor("g_row", [1, D], F32))
        emb = ctx.enter_context(nc.sbuf_tensor("emb", [1, DD], F32))
        a_row = ctx.enter_context(nc.sbuf_tensor("a_row", [1, D], F32))
        a_bc = ctx.enter_context(nc.sbuf_tensor("a_bc", [P, D], BF16))
        b_bc = ctx.enter_context(nc.sbuf_tensor("b_bc", [P, D], BF16))

        xt = [
            ctx.enter_context(nc.sbuf_tensor(f"xt{j}", [P, D], F32))
            for j in range(NX)
        ]
        zt = [
            ctx.enter_context(nc.sbuf_tensor(f"zt{j}", [P, D], BF16))
            for j in range(NZ)
        ]
        sq = ctx.enter_context(nc.sbuf_tensor("sqs", [P, D], F32))
        ss = [
            ctx.enter_context(nc.sbuf_tensor(f"ss{j}", [P, 1], F32))
            for j in range(NS)
        ]
        std = [
            ctx.enter_context(nc.sbuf_tensor(f"std{j}", [P, 1], F32))
            for j in range(NS)
        ]
        inv = [
            ctx.enter_context(nc.sbuf_tensor(f"inv{j}", [P, 1], F32))
            for j in range(NS)
        ]

        par_ps = ctx.enter_context(nc.psum_tensor("par_ps", [P, 2], F32))
        emb_ps = ctx.enter_context(nc.psum_tensor("emb_ps", [1, 4, 512], F32))
        bc_ps = [
            ctx.enter_context(nc.psum_tensor(f"bc_ps{j}", [P, 512], F32))
            for j in range(2)
        ]

        # ---------------- semaphores ----------------
        sem_pre = ctx.enter_context(nc.semaphore("pre"))
        sem_vec = ctx.enter_context(nc.semaphore("vec"))
        sem_wk = ctx.enter_context(nc.semaphore("wk"))
        sem_xt = ctx.enter_context(nc.semaphore("xts"))
        sem_ot = ctx.enter_context(nc.semaphore("ots"))
        sem_r127 = ctx.enter_context(nc.semaphore("r127s"))
        sem_r127cp = ctx.enter_context(nc.semaphore("r127cp"))
        sem_pe1 = ctx.enter_context(nc.semaphore("pe1"))
        sem_cq = ctx.enter_context(nc.semaphore("cqs"))
        sem_redA = ctx.enter_context(nc.semaphore("redA"))
        sem_sw = ctx.enter_context(nc.semaphore("sws"))
        sem_qrdy = ctx.enter_context(nc.semaphore("qrdy"))
        sem_mg = ctx.enter_context(nc.semaphore("mg"))
        sem_wq = ctx.enter_context(nc.semaphore("wq"))
        sem_mmk = ctx.enter_context(nc.semaphore("mmk"))
        sem_emb = ctx.enter_context(nc.semaphore("embs"))
        sem_embB = ctx.enter_context(nc.semaphore("embB"))
        sem_bcmm = ctx.enter_context(nc.semaphore("bcmm"))
        sem_bccp = ctx.enter_context(nc.semaphore("bccp"))
        sem_std = ctx.enter_context(nc.semaphore("stds"))
        sem_zA = ctx.enter_context(nc.semaphore("zA"))
        sem_add = ctx.enter_context(nc.semaphore("adds"))

        wt_r = wt_d[:].rearrange("(k p) n -> k p n", p=P)

        with nc.Block() as block:

            # ================= SP =================
            @block.sync
            def _(sync):
                # weight chunks first: full DMA priority AND first in the
                # SP descriptor-generation queue
                for k in range(KC):
                    sync.dma_start(out=wt_sb[:, k, :], in_=wt_r[k, :, :]).then_inc(
                        sem_wk, 16
                    )
                sync.dma_start(out=c_row[:], in_=c_d[None, :]).then_inc(sem_vec, 16)
                with nc.allow_non_contiguous_dma(reason="tiny 4KB c chunk load"):
                    sync.dma_start(
                        out=ct[:], in_=c_d[:].rearrange("(k p) -> p k", p=P)
                    ).then_inc(sem_vec, 16)
                sync.dma_start(out=b_row[:], in_=b_d[None, :]).then_inc(sem_vec, 16)
                sync.dma_start(out=g_row[:], in_=g_d[None, :]).then_inc(sem_vec, 16)
                for j in range(NX):
                    sync.dma_start(
                        out=xt[j][:], in_=x_d[j * P : (j + 1) * P, :]
                    ).then_inc(sem_xt, 16)
                for i in range(NT):
                    sync.wait_ge(sem_add, i + 1)
                    sync.dma_start(
                        out=out_d[i * P : (i + 1) * P, :], in_=xt[i % NX][:]
                    ).then_inc(sem_ot, 16)
                    if i + NX < NT:
                        j = i + NX
                        sync.dma_start(
                            out=xt[j % NX][:], in_=x_d[j * P : (j + 1) * P, :]
                        ).then_inc(sem_xt, 16)
                sync.wait_ge(sem_ot, 16 * NT)

            # ================= ACT =================
            @block.scalar
            def _(scalar):
                scalar.wait_ge(sem_pre, 2)
                # per-chunk |w| reduces as the weight DMAs land
                for k in range(KC):
                    scalar.wait_ge(sem_wk, 16 * (k + 1))
                    scalar.drain()
                    scalar.activation(
                        u[k % NU][:], wt_sb[:, k, :], ACTF.Abs,
                        accum_out=red[:, k : k + 1],
                    ).then_inc(sem_redA, 1)
                # magic-round passes for even chunks: u = bf16(w*swinv + 192)
                scalar.wait_ge(sem_qrdy, 1)
                for k in range(KC):
                    if not act_magic(k):
                        continue
                    if k >= NU:
                        scalar.wait_ge(sem_mmk, k - NU + 1)
                    scalar.drain()
                    scalar.activation(
                        u[k % NU][:], wt_sb[:, k, :], ACTF.Copy,
                        bias=MBF, scale=swinv[:],
                    ).then_inc(sem_mg, 1)
                # x statistics stream (+ z for ACT-owned tiles)
                for i in range(NT + 2):
                    if i < NT:
                        scalar.wait_ge(sem_xt, 16 * (i + 1))
                        if i >= NS:
                            scalar.wait_ge(sem_add, i - NS + 1)
                    scalar.drain()
                    if i < NT:
                        scalar.activation(
                            sq[:], xt[i % NX][:], ACTF.Square,
                            accum_out=ss[i % NS][:],
                        )
                    if 1 <= i <= NT:
                        scalar.activation(
                            std[(i - 1) % NS][:], ss[(i - 1) % NS][:], ACTF.Ln,
                            bias=eps_t[:], scale=1.0 / D,
                        )
                    if i >= 2:
                        j = i - 2
                        scalar.activation(
                            inv[j % NS][:], std[j % NS][:], ACTF.Exp,
                            scale=-0.5,
                        ).then_inc(sem_std, 1)
                        if act_owns_z(j):
                            if j >= NZ:
                                scalar.wait_ge(sem_add, j - NZ + 1)
                            scalar.drain()
                            scalar.activation(
                                zt[j % NZ][:], xt[j % NX][:], ACTF.Copy,
                                scale=inv[j % NS][:],
                            ).then_inc(sem_zA, 1)

            # ================= DVE =================
            @block.vector
            def _(vector):
                vector.memset(ones[:], 1.0).then_inc(sem_pre, 1)
                vector.memset(eps_t[:], EPS_RMS).then_inc(sem_pre, 1)

                # --- c quant ---
                vector.wait_ge(sem_vec, 64)
                vector.tensor_reduce(
                    out=am[:], in_=c_row[:], axis=mybir.AxisListType.X,
                    op=ALU.max, apply_absolute_value=True,
                )
                vector.drain()
                vector.tensor_scalar(
                    out=amc[:], in0=am[:], scalar1=EPS_Q, scalar2=None, op0=ALU.max
                )
                vector.drain()
                vector.reciprocal(rc[:], amc[:])
                vector.drain()
                vector.tensor_scalar(
                    out=r127[:], in0=rc[:], scalar1=127.0, scalar2=None,
                    op0=ALU.mult,
                ).then_inc(sem_r127, 1)
                vector.wait_ge(sem_pe1, 1)
                vector.tensor_copy(r127_b[:], par_ps[:, 1:2]).then_inc(sem_r127cp, 1)
                vector.drain()
                vector.tensor_scalar(
                    out=cqt[:], in0=ct[:], scalar1=r127_b[:], scalar2=MAGIC,
                    op0=ALU.mult, op1=ALU.add,
                )
                vector.drain()
                vector.tensor_scalar(
                    out=cqi[:], in0=cqt[:], scalar1=MAGIC, scalar2=None,
                    op0=ALU.subtract,
                ).then_inc(sem_cq, 1)

                # --- weight stats tail ---
                vector.wait_ge(sem_redA, KC)
                vector.tensor_reduce(
                    out=sw[:], in_=red[:], axis=mybir.AxisListType.X, op=ALU.add
                ).then_inc(sem_sw, 1)
                vector.wait_ge(sem_pe1, 2)
                vector.tensor_copy(swa[:], par_ps[:, 0:1])
                vector.drain()
                vector.tensor_scalar(
                    out=mp[:], in0=swa[:], scalar1=1.0 / (CD * DD), scalar2=EPS_Q,
                    op0=ALU.mult, op1=ALU.max,
                )
                vector.drain()
                vector.reciprocal(swinv[:], mp[:]).then_inc(sem_qrdy, 1)
                vector.drain()

                # --- output scale ---
                vector.tensor_tensor(osx[:], amc[:], mp[0:1, :], op=ALU.mult)
                vector.drain()
                vector.tensor_scalar(
                    out=os_t[:], in0=osx[:], scalar1=1.0 / 127.0, scalar2=None,
                    op0=ALU.mult,
                )

                # --- w quant: DVE magics (odd chunks) + clamp/sub all ---
                for k in range(KC):
                    if act_magic(k):
                        vector.wait_ge(sem_mg, n_mg(k))
                    else:
                        if k >= NU:
                            vector.wait_ge(sem_mmk, k - NU + 1)
                        vector.tensor_scalar(
                            out=u[k % NU][:], in0=wt_sb[:, k, :],
                            scalar1=swinv[:], scalar2=MBF,
                            op0=ALU.mult, op1=ALU.add,
                        )
                        vector.drain()
                    vector.tensor_scalar(
                        out=u[k % NU][:], in0=u[k % NU][:], scalar1=CHI,
                        scalar2=CLO, op0=ALU.min, op1=ALU.max,
                    )
                    vector.drain()
                    vector.tensor_scalar(
                        out=u[k % NU][:], in0=u[k % NU][:], scalar1=MBF,
                        scalar2=None, op0=ALU.subtract,
                    ).then_inc(sem_wq, 1)

                # --- emb epilogue (scale half first, then shift half) ---
                vector.wait_ge(sem_mmk, KC)
                vector.drain()
                vector.scalar_tensor_tensor(
                    out=emb[:, 0:512], in0=emb_ps[:, 0, :], scalar=os_t[:],
                    in1=b_row[:, 0:512], op0=ALU.mult, op1=ALU.add,
                )
                vector.scalar_tensor_tensor(
                    out=emb[:, 512:1024], in0=emb_ps[:, 1, :], scalar=os_t[:],
                    in1=b_row[:, 512:1024], op0=ALU.mult, op1=ALU.add,
                )
                vector.drain()
                vector.scalar_tensor_tensor(
                    out=a_row[:, 0:512], in0=emb[:, 0:512], scalar=1.0,
                    in1=g_row[:, 0:512], op0=ALU.add, op1=ALU.mult,
                ).then_inc(sem_emb, 1)
                vector.scalar_tensor_tensor(
                    out=a_row[:, 512:1024], in0=emb[:, 512:1024], scalar=1.0,
                    in1=g_row[:, 512:1024], op0=ALU.add, op1=ALU.mult,
                ).then_inc(sem_emb, 1)
                vector.scalar_tensor_tensor(
                    out=emb[:, 1024:1536], in0=emb_ps[:, 2, :], scalar=os_t[:],
                    in1=b_row[:, 1024:1536], op0=ALU.mult, op1=ALU.add,
                ).then_inc(sem_embB, 1)
                vector.scalar_tensor_tensor(
                    out=emb[:, 1536:2048], in0=emb_ps[:, 3, :], scalar=os_t[:],
                    in1=b_row[:, 1536:2048], op0=ALU.mult, op1=ALU.add,
                ).then_inc(sem_embB, 1)

                # --- broadcast copies PSUM -> SBUF (bf16) ---
                vector.wait_ge(sem_bcmm, 1)
                vector.tensor_copy(a_bc[:, 0:512], bc_ps[0][:]).then_inc(sem_bccp, 1)
                vector.wait_ge(sem_bcmm, 2)
                vector.tensor_copy(a_bc[:, 512:1024], bc_ps[1][:]).then_inc(
                    sem_bccp, 1
                )
                vector.wait_ge(sem_bcmm, 3)
                vector.tensor_copy(b_bc[:, 0:512], bc_ps[0][:])
                vector.wait_ge(sem_bcmm, 4)
                vector.tensor_copy(b_bc[:, 512:1024], bc_ps[1][:])
                vector.drain()

                # --- x modulation stream ---
                for i in range(NT):
                    if act_owns_z(i):
                        vector.wait_ge(sem_zA, n_zA(i))
                    else:
                        vector.wait_ge(sem_std, i + 1)
                        if i >= NZ:
                            vector.wait_ge(sem_add, i - NZ + 1)
                        vector.tensor_scalar(
                            out=zt[i % NZ][:], in0=xt[i % NX][:],
                            scalar1=inv[i % NS][:], scalar2=None, op0=ALU.mult,
                        )
                        vector.drain()
                    vector.tensor_tensor(
                        out=zt[i % NZ][:], in0=zt[i % NZ][:], in1=a_bc[:],
                        op=ALU.mult,
                    )
                    vector.drain()
                    vector.tensor_tensor(
                        out=xt[i % NX][:], in0=zt[i % NZ][:], in1=b_bc[:],
                        op=ALU.add,
                    ).then_inc(sem_add, 1)

            # ================= PE =================
            @block.tensor
            def _(tensor):
                tensor.wait_ge(sem_pre, 1)
                tensor.wait_ge(sem_r127, 1)
                tensor.matmul(
                    par_ps[:, 1:2], lhsT=ones[0:1, :], rhs=r127[:],
                    start=True, stop=True,
                ).then_inc(sem_pe1, 1)
                tensor.wait_ge(sem_r127cp, 1)
                tensor.wait_ge(sem_sw, 1)
                tensor.matmul(
                    par_ps[:, 0:1], lhsT=ones[:], rhs=sw[:], start=True, stop=True
                ).then_inc(sem_pe1, 1)
                tensor.wait_ge(sem_cq, 1)
                for k in range(KC):
                    tensor.wait_ge(sem_wq, k + 1)
                    for n in range(4):
                        mmi = tensor.matmul(
                            emb_ps[:, n, :],
                            lhsT=cqi[:, k : k + 1],
                            rhs=u[k % NU][:, n * 512 : (n + 1) * 512],
                            start=(k == 0),
                            stop=(k == KC - 1),
                        )
                        if n == 3:
                            mmi.then_inc(sem_mmk, 1)
                tensor.wait_ge(sem_emb, 1)
                tensor.matmul(
                    bc_ps[0][:], lhsT=ones[0:1, :], rhs=a_row[:, 0:512],
                    start=True, stop=True,
                ).then_inc(sem_bcmm, 1)
                tensor.wait_ge(sem_emb, 2)
                tensor.matmul(
                    bc_ps[1][:], lhsT=ones[0:1, :], rhs=a_row[:, 512:1024],
                    start=True, stop=True,
                ).then_inc(sem_bcmm, 1)
                tensor.wait_ge(sem_bccp, 2)
                tensor.wait_ge(sem_embB, 1)
                tensor.matmul(
                    bc_ps[0][:], lhsT=ones[0:1, :], rhs=emb[:, D : D + 512],
                    start=True, stop=True,
                ).then_inc(sem_bcmm, 1)
                tensor.wait_ge(sem_embB, 2)
                tensor.matmul(
                    bc_ps[1][:], lhsT=ones[0:1, :], rhs=emb[:, D + 512 : DD],
                    start=True, stop=True,
                ).then_inc(sem_bcmm, 1)

    return nc


_CACHE = {}


def _built(S=S_FULL):
    key = ("nc", S)
    if key not in _CACHE:
        _CACHE[key] = build(S)
    return _CACHE[key]


def kernel(x, c, w_proj, b_proj, rms_weight, _trace=False):
    x = np.ascontiguousarray(np.asarray(x, dtype=np.float32))
    c = np.ascontiguousarray(np.asarray(c, dtype=np.float32))
    w_proj = np.asarray(w_proj, dtype=np.float32)
    b_proj = np.ascontiguousarray(np.asarray(b_proj, dtype=np.float32))
    rms_weight = np.ascontiguousarray(np.asarray(rms_weight, dtype=np.float32))

    nc = _built(x.shape[1])
    wt = np.ascontiguousarray(w_proj.T)

    in_maps = [
        {"x": x[i], "c": c[i], "wt": wt, "b": b_proj, "g": rms_weight}
        for i in range(B)
    ]
    res = run_bass_kernel_spmd(nc, in_maps, list(range(B)), trace=_trace)
    kernel.last_results = res
    kernel.last_exec_time_ns = res.exec_time_ns
    return np.stack([res.results[i]["out"] for i in range(B)], axis=0)
